# revision 1
# baseline (speedup 1.0000x reference)
"""Trainium2 Bass kernel for 16-head MultiHeadAttention (B=2, S=2048, D=1024, f32).

Sharding: 8 cores = 2 (batch) x 4 (head groups of 4 heads).
Each core gets a col-shard of Wq/Wk/Wv ([1024,256]) + row-shard of Wo ([256,1024]),
computes a full [2048,1024] partial output split across two DRAM tensors
(one per head-pair); the host sums the 16 partials into [2,2048,1024].

On-device pipeline (transposed layouts, seq on the free axis):
  QT/KT = Wpair^T @ x^T             -> [128(=2 heads x 64), 2048] f32r
  VT    = Wv_pair^T @ xv^T, then PE-transposed to V_aug [j, head, 65]
          (65th column = ones so AV emits softmax denominators for free)
  sT    = KT_h^T-slice @ QT_h-slice, two heads row-packed in the PE array
          concurrently via tile_position (0,0)/(64,0)
  expT  = exp(0.125 * sT) via ACT from PSUM [128,1024] spans -> f32r
  outT/rowsum = V_aug^T @ expT          (M=65: rows 0-63 outT, row 64 rowsum)
  per-chunk: rowsum row -> DMA partition-gather -> lane-parallel reciprocal
          -> K=1 ones-matmul broadcast -> multiplied into outT (f32r)
  partial = outT^T @ Wo_shard           (K=128 over stacked head pairs),
          emitted per 256-row group as soon as its outT columns are ready.

All matmuls run as float32r (TF32-like 11-bit mantissa): full PE speed at
near-fp32 accuracy. Host pre-rounds inputs to the fp32r grid.
"""

import sys

import numpy as np

if "/opt/trn_rl_repo" not in sys.path:
    sys.path.insert(0, "/opt/trn_rl_repo")

import concourse.bacc as bacc
import concourse.mybir as mybir
import concourse.tile as tile
from concourse.masks import make_identity

F32 = mybir.dt.float32
F32R = mybir.dt.float32r

B, S, D, H = 2, 2048, 1024, 16
DK = D // H          # 64
HL = 4               # heads per core
DG = HL * DK         # 256
SCALE = 0.125        # 1/sqrt(DK)

ET = D // 128        # 8 e-tiles
JT = S // 128        # 16 j-tiles
QC = S // 512        # 4 q-chunks


def _round_fp32r(x: np.ndarray) -> np.ndarray:
    """Round f32 to the fp32r grid (11-bit mantissa, RNE), like walrus fp32_to_fp32r."""
    u = x.view(np.uint32).astype(np.uint64)
    u = (u + 0x7FF + ((u >> 12) & 1)) & np.uint64(0xFFFFF000)
    return u.astype(np.uint32).view(np.float32)


def _build_nc():
    nc = bacc.Bacc("TRN2", target_bir_lowering=False, debug=False)

    xq = nc.dram_tensor("xq", [D, S], F32, kind="ExternalInput").ap()
    xk = nc.dram_tensor("xk", [D, S], F32, kind="ExternalInput").ap()
    xv = nc.dram_tensor("xv", [D, S], F32, kind="ExternalInput").ap()
    wq = nc.dram_tensor("wq", [D, DG], F32, kind="ExternalInput").ap()
    wk = nc.dram_tensor("wk", [D, DG], F32, kind="ExternalInput").ap()
    wv = nc.dram_tensor("wv", [D, DG], F32, kind="ExternalInput").ap()
    wo = nc.dram_tensor("wo", [DG, D], F32, kind="ExternalInput").ap()
    out = nc.dram_tensor("out", [S, D], F32, kind="ExternalOutput").ap()
    out2 = nc.dram_tensor("out2", [S, D], F32, kind="ExternalOutput").ap()

    with tile.TileContext(nc) as tc:
        with (
            tc.tile_pool(name="wpool", bufs=1) as wpool,
            tc.tile_pool(name="xin", bufs=3) as xin,
            tc.tile_pool(name="proj", bufs=1) as proj,
            tc.tile_pool(name="expp", bufs=3) as expp,
            tc.tile_pool(name="stp", bufs=5) as stp,
            tc.tile_pool(name="work", bufs=2) as work,
            tc.tile_pool(name="small", bufs=2) as small,
        ):
            # ---- constants + early weights (wk/wq on the fast queues) ------
            wk_sb = [wpool.tile([128, DG], F32R, tag=f"wk{e}", name=f"wk{e}")
                     for e in range(ET)]
            wq_sb = [wpool.tile([128, DG], F32R, tag=f"wq{e}", name=f"wq{e}")
                     for e in range(ET)]
            wv_sb = [wpool.tile([128, DG], F32R, tag=f"wv{e}", name=f"wv{e}")
                     for e in range(ET)]

            ones16 = wpool.tile([16, 64], F32, tag="ones16", name="ones16")
            nc.vector.memset(ones16, 1.0)
            ones_r = wpool.tile([16, 64], F32R, tag="ones_r", name="ones_r")
            nc.vector.tensor_copy(ones_r, ones16)
            ones_col = wpool.tile([128, 64], F32, tag="ones_col", name="ones_col")
            nc.vector.memset(ones_col, 1.0)
            ident_f = wpool.tile([128, 128], F32, tag="ident_f", name="ident_f")
            make_identity(nc, ident_f)
            ident = wpool.tile([128, 128], F32R, tag="ident", name="ident")
            nc.vector.tensor_copy(ident, ident_f)

            # ---- persistent activation tiles -------------------------------
            kt_sb = [proj.tile([128, S], F32R, tag=f"kt{p}", name=f"kt{p}")
                     for p in range(2)]
            qt_sb = [proj.tile([128, S], F32R, tag=f"qt{p}", name=f"qt{p}")
                     for p in range(2)]
            v_sb = [proj.tile([128, JT // 2, HL, DK + 1], F32R,
                              tag=f"v{hh}", name=f"v{hh}") for hh in range(2)]

            # ---- phase A: projections (8 PSUM accumulators, x streamed) ----
            with tc.tile_pool(name="ps_a", bufs=1, space="PSUM") as ps_a:
                QS = (nc.sync, nc.scalar, nc.gpsimd)

                def proj_pairs(x_dram, w_tiles, dst_tiles, nm, w_dram):
                    accs = [ps_a.tile([128, 512], F32, tag="pa", bufs=8,
                                      name=f"acc_{nm}_{p}_{c}")
                            for p in range(2) for c in range(QC)]
                    # DMA emission order = per-queue service order: lead with
                    # the first x tiles, then the (small) weight tiles, then
                    # the remaining x tiles, round-robin across 3 queues.
                    xt, rr = [None] * ET, 0

                    def load_x(e):
                        nonlocal rr
                        xt[e] = xin.tile([128, S], F32R, tag="xs", name=f"x_{nm}{e}")
                        QS[rr % 3].dma_start(
                            xt[e], x_dram.bitcast(F32R)[e * 128:(e + 1) * 128, :])
                        rr += 1

                    for e in range(3):
                        load_x(e)
                    for e in range(ET):
                        QS[(rr + e) % 3].dma_start(
                            w_tiles[e], w_dram.bitcast(F32R)[e * 128:(e + 1) * 128, :])
                    for e in range(3, ET):
                        load_x(e)
                    for e in range(ET):
                        for p in range(2):
                            for c in range(QC):
                                nc.tensor.matmul(
                                    accs[p * QC + c],
                                    w_tiles[e][:, p * 128:(p + 1) * 128],
                                    xt[e][:, c * 512:(c + 1) * 512],
                                    start=(e == 0), stop=(e == ET - 1),
                                )
                    for p in range(2):
                        for c in range(QC):
                            nc.vector.tensor_copy(
                                dst_tiles[p][:, c * 512:(c + 1) * 512],
                                accs[p * QC + c],
                            )

                proj_pairs(xk, wk_sb, kt_sb, "k", wk)
                proj_pairs(xq, wq_sb, qt_sb, "q", wq)
                vt_sb = [proj.tile([128, S], F32R, tag=f"ot{p}", name=f"vt{p}")
                         for p in range(2)]
                proj_pairs(xv, wv_sb, vt_sb, "v", wv)

            # wo needed from mid-phase-B on; load during phase A tail
            wo_sb = [wpool.tile([128, D], F32R, tag=f"wo{p}", name=f"wo{p}")
                     for p in range(2)]
            for p in range(2):
                nc.gpsimd.dma_start(wo_sb[p], wo.bitcast(F32R)[p * 128:(p + 1) * 128, :])

            # V_aug via PE transpose of VT
            with tc.tile_pool(name="ps_t", bufs=2, space="PSUM") as ps_t:
                for p in range(2):
                    for jt in range(JT):
                        pt = ps_t.tile([128, 128], F32R, tag="pt", name=f"pt{p}_{jt}")
                        nc.tensor.transpose(
                            pt, vt_sb[p][:, jt * 128:(jt + 1) * 128], ident)
                        hh, j2 = divmod(jt, JT // 2)
                        nc.vector.tensor_copy(
                            v_sb[hh][:, j2, 2 * p:2 * p + 2, 0:DK],
                            pt.rearrange("j (h d) -> j h d", h=2),
                        )
                for hh in range(2):
                    nc.vector.tensor_copy(
                        v_sb[hh][:, :, :, DK:DK + 1],
                        ones_col[:, 0:32].rearrange("p (a b) -> p a b", a=8)[:, :, :, None],
                    )

            # ---- phases B+C: attention + output projection -----------------
            outt_sb = [proj.tile([128, S], F32R, tag=f"ot{p}", name=f"outt{p}")
                       for p in range(2)]

            with (
                tc.tile_pool(name="ps_sc", bufs=2, space="PSUM") as ps_sc,
                tc.tile_pool(name="ps_av", bufs=2, space="PSUM") as ps_av,
                tc.tile_pool(name="ps_wo", bufs=2, space="PSUM") as ps_wo,
            ):
                def wo_block(p, qg):
                    # partial += outT_p^T @ Wo_p for q-tiles 2qg, 2qg+1
                    osb = work.tile([128, 2, 1024], F32, tag="osb",
                                    name=f"osb{p}_{qg}")
                    for qq in range(2):
                        qt = qg * 2 + qq
                        for ch in range(2):
                            acc = ps_wo.tile([128, 512], F32, tag="po",
                                             name=f"po{p}_{qt}_{ch}")
                            nc.tensor.matmul(
                                acc,
                                outt_sb[p][:, qt * 128:(qt + 1) * 128],
                                wo_sb[p][:, ch * 512:(ch + 1) * 512],
                                start=True, stop=True,
                            )
                            nc.vector.tensor_copy(
                                osb[:, qq, ch * 512:(ch + 1) * 512], acc)
                    dst = (out if p == 0 else out2)[
                        qg * 256:(qg + 1) * 256, :].rearrange(
                        "(a j) e -> j a e", a=2)
                    nc.sync.dma_start(dst, osb)

                pending_norm = []
                for p in range(2):
                    hA, hB = 2 * p, 2 * p + 1
                    for c in range(QC):
                        csl = slice(c * 512, (c + 1) * 512)
                        # scores + exp, row-packed head pairs, 1 jt per group
                        exp_tiles = []
                        for jt in range(JT):
                            jsl = slice(jt * 128, (jt + 1) * 128)
                            sc = ps_sc.tile([128, 2 * 512], F32, tag="sc",
                                            name=f"sc{p}_{c}_{jt}")
                            nc.tensor.matmul(
                                sc[:, 0:512],
                                kt_sb[p][0:64, jsl],
                                qt_sb[p][0:64, csl],
                                start=True, stop=True,
                                tile_position=(0, 0),
                            )
                            nc.tensor.matmul(
                                sc[:, 512:1024],
                                kt_sb[p][64:128, jsl],
                                qt_sb[p][64:128, csl],
                                start=True, stop=True,
                                tile_position=(64, 0),
                            )
                            ex = expp.tile([128, 2, 512], F32R, tag="ex",
                                           name=f"ex{p}_{c}_{jt}")
                            nc.scalar.activation(
                                out=ex,
                                in_=sc.rearrange("j (t q) -> j t q", t=2),
                                func=mybir.ActivationFunctionType.Exp,
                                scale=SCALE,
                            )
                            exp_tiles.append(ex)
                        # deferred from previous chunk: normalization
                        # matmuls + Wo (the serial recip chain has finished
                        # by now, so the in-order PE stream won't stall)
                        for fn in pending_norm:
                            fn()
                        pending_norm = []
                        # AV for both heads (full-row K=128 accumulation)
                        avA = ps_av.tile([128, 512], F32, tag="av", name=f"avA{p}_{c}")
                        avB = ps_av.tile([128, 512], F32, tag="av", name=f"avB{p}_{c}")
                        for jt in range(JT):
                            hh, j2 = divmod(jt, JT // 2)
                            ex = exp_tiles[jt]
                            nc.tensor.matmul(
                                avA[0:DK + 1, :],
                                v_sb[hh][:, j2, hA, :],
                                ex[:, 0, :],
                                start=(jt == 0), stop=(jt == JT - 1),
                            )
                            nc.tensor.matmul(
                                avB[0:DK + 1, :],
                                v_sb[hh][:, j2, hB, :],
                                ex[:, 1, :],
                                start=(jt == 0), stop=(jt == JT - 1),
                            )
                        # stage outT + rowsum, run the recip chain now
                        # (DVE/DMA work, overlaps the next chunk's scores)
                        for i, av in ((0, avA), (1, avB)):
                            st = stp.tile([128, 512], F32, tag="st",
                                          name=f"st{p}_{c}_{i}")
                            nc.vector.tensor_copy(st[0:DK + 1, :], av[0:DK + 1, :])
                            rsg = small.tile([4, 128], F32, tag=f"rsg{i}",
                                             name=f"rsg{p}_{c}_{i}")
                            nc.sync.dma_start(
                                rsg,
                                st[DK:DK + 1, :].rearrange(
                                    "one (pp f) -> one pp f", pp=4),
                            )
                            nc.vector.reciprocal(rsg, rsg)
                            rrow = small.tile([1, 512], F32, tag=f"rrow{i}",
                                              name=f"rrow{p}_{c}_{i}")
                            nc.sync.dma_start(
                                rrow.rearrange("one (pp f) -> one pp f", pp=4),
                                rsg,
                            )
                            rbc = small.tile([64, 512], F32, tag=f"rbc{i}",
                                             name=f"rbc{p}_{c}_{i}")
                            nc.gpsimd.partition_broadcast(rbc, rrow)
                            nc.vector.tensor_tensor(
                                outt_sb[p][slice(i * 64, (i + 1) * 64), csl],
                                st[0:DK, :],
                                rbc,
                                mybir.AluOpType.mult,
                            )

                        def norm_and_wo(p=p, c=c):
                            wo_block(p, 2 * c)
                            wo_block(p, 2 * c + 1)

                        pending_norm = [norm_and_wo]

                for fn in pending_norm:
                    fn()

    nc.compile()
    return nc


_NC = None


def _get_nc():
    global _NC
    if _NC is None:
        _NC = _build_nc()
    return _NC


def make_in_maps(query, key, value, Wq, Wk, Wv, Wo):
    query = _round_fp32r(np.ascontiguousarray(query, dtype=np.float32))
    key_ = _round_fp32r(np.ascontiguousarray(key, dtype=np.float32))
    value = _round_fp32r(np.ascontiguousarray(value, dtype=np.float32))
    Wq = _round_fp32r(np.ascontiguousarray(Wq, dtype=np.float32))
    Wk = _round_fp32r(np.ascontiguousarray(Wk, dtype=np.float32))
    Wv = _round_fp32r(np.ascontiguousarray(Wv, dtype=np.float32))
    Wo = _round_fp32r(np.ascontiguousarray(Wo, dtype=np.float32))

    xqT = [np.ascontiguousarray(query[b].T) for b in range(B)]
    xkT = [np.ascontiguousarray(key_[b].T) for b in range(B)]
    xvT = [np.ascontiguousarray(value[b].T) for b in range(B)]

    in_maps = []
    for core in range(8):
        b, g = divmod(core, 4)
        sl = slice(g * DG, (g + 1) * DG)
        in_maps.append({
            "xq": xqT[b],
            "xk": xkT[b],
            "xv": xvT[b],
            "wq": np.ascontiguousarray(Wq[:, sl]),
            "wk": np.ascontiguousarray(Wk[:, sl]),
            "wv": np.ascontiguousarray(Wv[:, sl]),
            "wo": np.ascontiguousarray(Wo[sl, :]),
        })
    return in_maps


def combine_results(results):
    out = np.zeros((B, S, D), dtype=np.float32)
    for core in range(8):
        out[core // 4] += results[core]["out"]
        out[core // 4] += results[core]["out2"]
    return out


def kernel(query, key, value, Wq, Wk, Wv, Wo, _trace=False):
    from concourse import bass_utils

    nc = _get_nc()
    in_maps = make_in_maps(query, key, value, Wq, Wk, Wv, Wo)
    r = bass_utils.run_bass_kernel_spmd(
        nc, in_maps, core_ids=list(range(8)), trace=_trace
    )
    kernel.last_results = r
    return combine_results(r.results)



# revision 8
# speedup vs baseline: 1.1054x; 1.1054x over previous
"""Trainium2 Bass kernel for 16-head MultiHeadAttention (B=2, S=2048, D=1024).

Sharding: 8 cores = 2 (batch) x 4 (head groups of 4 heads).
Each core gets a col-shard of Wq/Wk/Wv ([1024,256]) + row-shard of Wo
([256,1024]) and emits ONE full [2048,1024] bf16 partial (K=256 PSUM
accumulation over the two head-pairs); the host sums 4 partials per batch.

All SBUF tensors are bf16 (PSUM accumulation stays f32); inputs are
converted to bf16 on the host, halving DMA traffic. Measured bf16
end-to-end pipeline error ~6e-3 (gate 2e-2).

Engine plan (the previous f32r version measured 308us with the PE
HAM-throttled to 1.2 GHz for half the kernel due to per-chunk exp waits):
 - phase B is emitted as 8 slots of (head-pair p, q-chunk c), with the
   scores MMs of slot s interleaved 1:1 with the AV MMs of slot s-1 at
   j-tile granularity, plus filler PE work (chunked Q projection,
   V projection in [j,dg] layout -- no PE transposes -- and the Wo
   matmuls) so the PE never idles long enough for HAM to re-throttle.
 - exp runs on ACT (~135us serial, the co-bottleneck) reading PSUM score
   tiles ping-ponged at j-tile granularity (2 banks x 2).
 - PSUM budget: sc 2x[128,2,512] (4 banks) + av 2x[128,512] + fill
   2x[128,512] = 8 banks exactly.
 - rowsums via the ones-column in V_aug (M=65 AV matmuls); per-q-chunk
   reciprocal + partition_broadcast + multiply on DVE/Pool.
"""

import sys

import numpy as np

if "/opt/trn_rl_repo" not in sys.path:
    sys.path.insert(0, "/opt/trn_rl_repo")

import ml_dtypes

import concourse.bacc as bacc
import concourse.mybir as mybir
import concourse.tile as tile

F32 = mybir.dt.float32
BF16 = mybir.dt.bfloat16
NPBF = ml_dtypes.bfloat16

B, S, D, H = 2, 2048, 1024, 16
DK = D // H          # 64
HL = 4               # heads per core
DG = HL * DK         # 256
SCALE = 0.125        # 1/sqrt(DK)

ET = D // 128        # 8 e-tiles (contraction tiles for projections)
JT = S // 128        # 16 j-tiles (key positions)
QC = 4               # q-chunks of 512 per head-pair
EXP = mybir.ActivationFunctionType.Exp
MULT = mybir.AluOpType.mult


def _build_nc():
    nc = bacc.Bacc("TRN2", target_bir_lowering=False, debug=False)

    xq = nc.dram_tensor("xq", [D, S], BF16, kind="ExternalInput").ap()
    xk = nc.dram_tensor("xk", [D, S], BF16, kind="ExternalInput").ap()
    xv = nc.dram_tensor("xv", [D, S], BF16, kind="ExternalInput").ap()
    wq = nc.dram_tensor("wq", [D, DG], BF16, kind="ExternalInput").ap()
    wk = nc.dram_tensor("wk", [D, DG], BF16, kind="ExternalInput").ap()
    wv = nc.dram_tensor("wv", [D, DG], BF16, kind="ExternalInput").ap()
    wo = nc.dram_tensor("wo", [DG, D], BF16, kind="ExternalInput").ap()
    out = nc.dram_tensor("out", [S, D], BF16, kind="ExternalOutput").ap()

    with tile.TileContext(nc) as tc:
        with (
            tc.tile_pool(name="wpool", bufs=1) as wpool,
            tc.tile_pool(name="xkp", bufs=3) as xkp,
            tc.tile_pool(name="xvp", bufs=1) as xvp,
            tc.tile_pool(name="xqp", bufs=2) as xqp,
            tc.tile_pool(name="proj", bufs=1) as proj,
            tc.tile_pool(name="expp", bufs=20) as expp,
            tc.tile_pool(name="outtp", bufs=4) as outtp,
            tc.tile_pool(name="osbp", bufs=2) as osbp,
            tc.tile_pool(name="smallp", bufs=3) as smallp,
        ):
            # ---- weights + persistent activation tiles ---------------------
            wk_sb = [wpool.tile([128, DG], BF16, tag=f"wk{e}", name=f"wk{e}")
                     for e in range(ET)]
            wq_sb = [wpool.tile([128, DG], BF16, tag=f"wq{e}", name=f"wq{e}")
                     for e in range(ET)]
            wv_sb = [wpool.tile([128, DG], BF16, tag=f"wv{e}", name=f"wv{e}")
                     for e in range(ET)]
            wo_sb = [wpool.tile([128, D], BF16, tag=f"wo{p}", name=f"wo{p}")
                     for p in range(2)]

            kt = [proj.tile([128, S], BF16, tag=f"kt{p}", name=f"kt{p}")
                  for p in range(2)]
            qt = [proj.tile([128, S], BF16, tag=f"qt{p}", name=f"qt{p}")
                  for p in range(2)]
            # V in [j, head, dk+1] layout; the 65th column of ones makes the
            # AV matmul emit softmax denominators in PSUM row 64.
            v_sb = proj.tile([128, JT, HL, DK + 1], BF16, tag="v", name="v_sb")
            nc.vector.memset(v_sb[:, :, :, DK:DK + 1], 1.0)

            # Prime the ACT exp table set during the prologue so the first
            # real exp doesn't eat the ~2.7us ACT_TABLE_LOAD mid-pipeline.
            prime = wpool.tile([1, 8], F32, tag="prime", name="prime")
            nc.vector.memset(prime, 0.0)
            nc.scalar.activation(out=prime, in_=prime, func=EXP, scale=1.0)

            QS = (nc.sync, nc.scalar, nc.gpsimd)

            # ---- DMA emission ----------------------------------------------
            # xk leads (K-proj gates everything); wk tiles slip in early on
            # the queue-robin; then the c0/c1 xq slabs, then xv, then the
            # rest. Per-queue order == service order.
            xk_t = [None] * ET
            rr = 0

            def load_xk(e):
                nonlocal rr
                xk_t[e] = xkp.tile([128, S], BF16, tag="xk", name=f"xk{e}")
                QS[rr % 3].dma_start(xk_t[e], xk[e * 128:(e + 1) * 128, :])
                rr += 1

            for e in range(3):
                load_xk(e)
            for e in range(ET):
                QS[(rr + e) % 3].dma_start(
                    wk_sb[e], wk[e * 128:(e + 1) * 128, :])
            for e in range(3, ET):
                load_xk(e)
            for e in range(ET):
                QS[(rr + e) % 3].dma_start(
                    wq_sb[e], wq[e * 128:(e + 1) * 128, :])

            # xq slabs: [128, ET, 512] per chunk, written by 8 slice-DMAs.
            xq_slab = [None] * QC

            def load_xq(c, queues=(0, 1, 2)):
                xq_slab[c] = xqp.tile([128, ET, 512], BF16, tag="xqc",
                                      name=f"xq{c}")
                for e in range(ET):
                    QS[queues[e % len(queues)]].dma_start(
                        xq_slab[c][:, e, :],
                        xq[e * 128:(e + 1) * 128, c * 512:(c + 1) * 512])

            load_xq(0)
            xv_t = [None] * ET
            for e in range(ET):
                xv_t[e] = xvp.tile([128, S], BF16, tag=f"xv{e}", name=f"xv{e}")
                QS[e % 3].dma_start(xv_t[e], xv[e * 128:(e + 1) * 128, :])
            for e in range(ET):
                QS[e % 3].dma_start(wv_sb[e], wv[e * 128:(e + 1) * 128, :])
            for p in range(2):
                QS[p % 3].dma_start(wo_sb[p], wo[p * 128:(p + 1) * 128, :])
            load_xq(1)

            # ---- prologue: K projection + Q chunk 0 ------------------------
            with tc.tile_pool(name="ps_a", bufs=8, space="PSUM") as ps_a:
                kacc = [ps_a.tile([128, 512], F32, tag="pa", name=f"kacc{i}")
                        for i in range(8)]
                for e in range(ET):
                    for p in range(2):
                        for cc in range(4):
                            nc.tensor.matmul(
                                kacc[p * 4 + cc],
                                wk_sb[e][:, p * 128:(p + 1) * 128],
                                xk_t[e][:, cc * 512:(cc + 1) * 512],
                                start=(e == 0), stop=(e == ET - 1),
                            )
                for p in range(2):
                    for cc in range(4):
                        nc.vector.tensor_copy(
                            kt[p][:, cc * 512:(cc + 1) * 512], kacc[p * 4 + cc])
                qacc = [ps_a.tile([128, 512], F32, tag="pa", name=f"qacc{p}")
                        for p in range(2)]
                for e in range(ET):
                    for p in range(2):
                        nc.tensor.matmul(
                            qacc[p],
                            wq_sb[e][:, p * 128:(p + 1) * 128],
                            xq_slab[0][:, e, :],
                            start=(e == 0), stop=(e == ET - 1),
                        )
                for p in range(2):
                    nc.vector.tensor_copy(qt[p][:, 0:512], qacc[p])

            # ---- phase B ---------------------------------------------------
            ex_tiles = {}
            av_state = {}
            outt = {}
            osb = {}

            with (
                tc.tile_pool(name="ps_sc", bufs=2, space="PSUM") as ps_sc,
                tc.tile_pool(name="ps_av", bufs=2, space="PSUM") as ps_av,
                tc.tile_pool(name="ps_fl", bufs=2, space="PSUM") as ps_fl,
            ):
                def emit_sc_jt(p, c, jt):
                    sc_t = ps_sc.tile([128, 2, 512], F32, tag="sc",
                                      name=f"sc{p}_{c}_{jt}")
                    csl = slice(c * 512, (c + 1) * 512)
                    jsl = slice(jt * 128, (jt + 1) * 128)
                    nc.tensor.matmul(sc_t[:, 0, :], kt[p][0:64, jsl],
                                     qt[p][0:64, csl], start=True, stop=True,
                                     tile_position=(0, 0))
                    nc.tensor.matmul(sc_t[:, 1, :], kt[p][64:128, jsl],
                                     qt[p][64:128, csl], start=True, stop=True,
                                     tile_position=(64, 0))
                    ex = expp.tile([128, 2, 512], BF16, tag="ex",
                                   name=f"ex{p}_{c}_{jt}")
                    nc.scalar.activation(out=ex, in_=sc_t, func=EXP,
                                         scale=SCALE)
                    ex_tiles[(p, c)].append(ex)

                def emit_av_jt(p, c, jt):
                    if jt == 0:
                        av_state[(p, c)] = (
                            ps_av.tile([128, 512], F32, tag="av",
                                       name=f"avA{p}_{c}"),
                            ps_av.tile([128, 512], F32, tag="av",
                                       name=f"avB{p}_{c}"),
                        )
                    avA, avB = av_state[(p, c)]
                    ex = ex_tiles[(p, c)][jt]
                    nc.tensor.matmul(avA[0:DK + 1, :], v_sb[:, jt, 2 * p, :],
                                     ex[:, 0, :],
                                     start=(jt == 0), stop=(jt == JT - 1))
                    nc.tensor.matmul(avB[0:DK + 1, :], v_sb[:, jt, 2 * p + 1, :],
                                     ex[:, 1, :],
                                     start=(jt == 0), stop=(jt == JT - 1))

                def emit_normalize(p, c):
                    avA, avB = av_state[(p, c)]
                    ot = outtp.tile([128, 512], BF16, tag="ot",
                                    name=f"ot{p}_{c}")
                    for i, av_ in ((0, avA), (1, avB)):
                        rs = smallp.tile([1, 512], F32, tag="rs",
                                         name=f"rs{p}_{c}_{i}")
                        nc.vector.tensor_copy(rs, av_[DK:DK + 1, :])
                        nc.vector.reciprocal(rs, rs)
                        rbc = smallp.tile([64, 512], F32, tag="rbc",
                                          name=f"rbc{p}_{c}_{i}")
                        nc.gpsimd.partition_broadcast(rbc, rs)
                        nc.vector.tensor_tensor(
                            ot[i * 64:(i + 1) * 64, :], av_[0:DK, :], rbc, MULT)
                    outt[(p, c)] = ot

                def emit_qproj(c):
                    qa = [ps_fl.tile([128, 512], F32, tag="fl",
                                     name=f"qa{c}_{p}") for p in range(2)]
                    for e in range(ET):
                        for p in range(2):
                            nc.tensor.matmul(
                                qa[p], wq_sb[e][:, p * 128:(p + 1) * 128],
                                xq_slab[c][:, e, :],
                                start=(e == 0), stop=(e == ET - 1))
                    for p in range(2):
                        nc.vector.tensor_copy(
                            qt[p][:, c * 512:(c + 1) * 512], qa[p])

                def emit_vq(i):
                    # quarter i: j-tiles 4i..4i+3 of V, projected [j, dg]
                    # via x-as-weights (no PE transposes needed). One j-tile
                    # per PSUM tile: a bank supports only one open
                    # accumulation group at a time.
                    for jj in range(4):
                        jt = 4 * i + jj
                        vt = ps_fl.tile([128, 512], F32, tag="fl",
                                        name=f"vq{i}_{jj}")
                        for e in range(ET):
                            nc.tensor.matmul(
                                vt[:, 0:DG],
                                xv_t[e][:, jt * 128:(jt + 1) * 128],
                                wv_sb[e],
                                start=(e == 0), stop=(e == ET - 1))
                        nc.vector.tensor_copy(
                            v_sb[:, jt, :, 0:DK],
                            vt[:, 0:DG].rearrange("p (h d) -> p h d", h=HL))

                def emit_wo_half(c, half):
                    if half == 0:
                        osb[c] = osbp.tile([128, 4, D], BF16, tag="osb",
                                           name=f"osb{c}")
                    ot0, ot1 = outt[(0, c)], outt[(1, c)]
                    for t4 in (2 * half, 2 * half + 1):
                        for ch in range(2):
                            wp = ps_fl.tile([128, 512], F32, tag="fl",
                                            name=f"wo{c}_{t4}_{ch}")
                            nc.tensor.matmul(
                                wp, ot0[:, t4 * 128:(t4 + 1) * 128],
                                wo_sb[0][:, ch * 512:(ch + 1) * 512],
                                start=True, stop=False)
                            nc.tensor.matmul(
                                wp, ot1[:, t4 * 128:(t4 + 1) * 128],
                                wo_sb[1][:, ch * 512:(ch + 1) * 512],
                                start=False, stop=True)
                            nc.vector.tensor_copy(
                                osb[c][:, t4, ch * 512:(ch + 1) * 512], wp)
                    if half == 1:
                        dst = out[c * 512:(c + 1) * 512, :].rearrange(
                            "(a j) e -> j a e", a=4)
                        nc.sync.dma_start(dst, osb[c])

                SLOTS = [(p, c) for c in range(QC) for p in range(2)]
                # Filler PE work per slot. Non-wo fillers go at jt 0/5/10
                # (they have no fresh dependencies); wo fillers go at jt 7/12
                # so the previous chunk's reciprocal chain (DVE/Pool) has
                # finished before the wo matmuls need outt.
                FILLERS = {
                    0: [("q", 1), ("v", 0), ("v", 1)],
                    1: [("v", 2), ("v", 3)],
                    2: [("q", 2)],
                    3: [("wo", 0, 0), ("wo", 0, 1)],
                    4: [("q", 3)],
                    5: [("wo", 1, 0)],
                    6: [("wo", 1, 1)],
                    7: [("wo", 2, 0), ("wo", 2, 1)],
                }

                def emit_filler(f):
                    if f[0] == "q":
                        emit_qproj(f[1])
                    elif f[0] == "v":
                        emit_vq(f[1])
                    else:
                        emit_wo_half(f[1], f[2])

                for s in range(9):
                    cur = SLOTS[s] if s < 8 else None
                    prev = SLOTS[s - 1] if s > 0 else None
                    fillers = list(FILLERS.get(s, []))
                    is_wo = fillers and fillers[0][0] == "wo"
                    spots = (7, 12) if is_wo else (0, 5, 10)
                    if cur is not None:
                        ex_tiles[cur] = []
                    # xq slab prefetch two chunks ahead (sync queue only;
                    # scalar queue must stay clear of waits once exp runs).
                    if s == 1:
                        load_xq(2, queues=(0,))
                    if s == 3:
                        load_xq(3, queues=(0,))
                    for jt in range(JT):
                        if cur is not None:
                            emit_sc_jt(*cur, jt)
                        if jt in spots and fillers:
                            emit_filler(fillers.pop(0))
                        # AV of the previous slot, shifted 2 j-tiles so the
                        # slot-start matmuls never wait on the previous
                        # slot's reciprocal chain freeing the av banks.
                        if prev is not None and jt >= 2:
                            emit_av_jt(*prev, jt - 2)
                    for f in fillers:
                        emit_filler(f)
                    if prev is not None:
                        emit_av_jt(*prev, 14)
                        emit_av_jt(*prev, 15)
                        emit_normalize(*prev)

                emit_wo_half(3, 0)
                emit_wo_half(3, 1)

    nc.compile()
    return nc


_NC = None


def _get_nc():
    global _NC
    if _NC is None:
        _NC = _build_nc()
    return _NC


def make_in_maps(query, key, value, Wq, Wk, Wv, Wo):
    query = np.ascontiguousarray(query, dtype=np.float32)
    key_ = np.ascontiguousarray(key, dtype=np.float32)
    value = np.ascontiguousarray(value, dtype=np.float32)
    xqT = [np.ascontiguousarray(query[b].T).astype(NPBF) for b in range(B)]
    xkT = [np.ascontiguousarray(key_[b].T).astype(NPBF) for b in range(B)]
    xvT = [np.ascontiguousarray(value[b].T).astype(NPBF) for b in range(B)]
    Wq = np.asarray(Wq, dtype=np.float32)
    Wk = np.asarray(Wk, dtype=np.float32)
    Wv = np.asarray(Wv, dtype=np.float32)
    Wo = np.asarray(Wo, dtype=np.float32)

    in_maps = []
    for core in range(8):
        b, g = divmod(core, 4)
        sl = slice(g * DG, (g + 1) * DG)
        in_maps.append({
            "xq": xqT[b],
            "xk": xkT[b],
            "xv": xvT[b],
            "wq": np.ascontiguousarray(Wq[:, sl]).astype(NPBF),
            "wk": np.ascontiguousarray(Wk[:, sl]).astype(NPBF),
            "wv": np.ascontiguousarray(Wv[:, sl]).astype(NPBF),
            "wo": np.ascontiguousarray(Wo[sl, :]).astype(NPBF),
        })
    return in_maps


def combine_results(results):
    out = np.zeros((B, S, D), dtype=np.float32)
    for core in range(8):
        out[core // 4] += np.asarray(results[core]["out"]).astype(np.float32)
    return out


def kernel(query, key, value, Wq, Wk, Wv, Wo, _trace=False):
    from concourse import bass_utils

    nc = _get_nc()
    in_maps = make_in_maps(query, key, value, Wq, Wk, Wv, Wo)
    r = bass_utils.run_bass_kernel_spmd(
        nc, in_maps, core_ids=list(range(8)), trace=_trace
    )
    kernel.last_results = r
    return combine_results(r.results)


# revision 11
# speedup vs baseline: 1.3536x; 1.2246x over previous
"""Trainium2 Bass kernel for 16-head MultiHeadAttention (B=2, S=2048, D=1024).

Sharding: 8 cores = 2 (batch) x 4 (head groups of 4 heads).
Each core gets a col-shard of Wq/Wk/Wv ([1024,256]) + row-shard of Wo
([256,1024]) and emits ONE full [2048,1024] bf16 partial (K=256 PSUM
accumulation over the two head-pairs); the host sums 4 partials per batch.

All SBUF tensors are bf16 (PSUM accumulation stays f32); inputs are
converted to bf16 on the host, halving DMA traffic. Measured bf16
end-to-end pipeline error ~6e-3 (gate 2e-2).

Engine plan (the previous f32r version measured 308us with the PE
HAM-throttled to 1.2 GHz for half the kernel due to per-chunk exp waits):
 - phase B is emitted as 8 slots of (head-pair p, q-chunk c), with the
   scores MMs of slot s interleaved 1:1 with the AV MMs of slot s-1 at
   j-tile granularity, plus filler PE work (chunked Q projection,
   V projection in [j,dg] layout -- no PE transposes -- and the Wo
   matmuls) so the PE never idles long enough for HAM to re-throttle.
 - exp runs on ACT (~135us serial, the co-bottleneck) reading PSUM score
   tiles ping-ponged at j-tile granularity (2 banks x 2).
 - PSUM budget: sc 2x[128,2,512] (4 banks) + av 2x[128,512] + fill
   2x[128,512] = 8 banks exactly.
 - rowsums via the ones-column in V_aug (M=65 AV matmuls); per-q-chunk
   reciprocal + partition_broadcast + multiply on DVE/Pool.
"""

import sys

import numpy as np

if "/opt/trn_rl_repo" not in sys.path:
    sys.path.insert(0, "/opt/trn_rl_repo")

import ml_dtypes

import concourse.bacc as bacc
import concourse.mybir as mybir
import concourse.tile as tile

F32 = mybir.dt.float32
BF16 = mybir.dt.bfloat16
NPBF = ml_dtypes.bfloat16

B, S, D, H = 2, 2048, 1024, 16
DK = D // H          # 64
HL = 4               # heads per core
DG = HL * DK         # 256
SCALE = 0.125        # 1/sqrt(DK)

ET = D // 128        # 8 e-tiles (contraction tiles for projections)
JT = S // 128        # 16 j-tiles (key positions)
QC = 4               # q-chunks of 512 per head-pair
EXP = mybir.ActivationFunctionType.Exp
MULT = mybir.AluOpType.mult


def _build_nc():
    nc = bacc.Bacc("TRN2", target_bir_lowering=False, debug=False)

    xq = nc.dram_tensor("xq", [D, S], BF16, kind="ExternalInput").ap()
    xk = nc.dram_tensor("xk", [D, S], BF16, kind="ExternalInput").ap()
    xv = nc.dram_tensor("xv", [D, S], BF16, kind="ExternalInput").ap()
    wq = nc.dram_tensor("wq", [D, DG], BF16, kind="ExternalInput").ap()
    wk = nc.dram_tensor("wk", [D, DG], BF16, kind="ExternalInput").ap()
    wv = nc.dram_tensor("wv", [D, DG], BF16, kind="ExternalInput").ap()
    wo = nc.dram_tensor("wo", [DG, D], BF16, kind="ExternalInput").ap()
    out = nc.dram_tensor("out", [S, D], BF16, kind="ExternalOutput").ap()

    with tile.TileContext(nc) as tc:
        with (
            tc.tile_pool(name="wpool", bufs=1) as wpool,
            tc.tile_pool(name="xkp", bufs=3) as xkp,
            tc.tile_pool(name="xvp", bufs=1) as xvp,
            tc.tile_pool(name="xqp", bufs=2) as xqp,
            tc.tile_pool(name="proj", bufs=1) as proj,
            tc.tile_pool(name="expp", bufs=20) as expp,
            tc.tile_pool(name="outtp", bufs=4) as outtp,
            tc.tile_pool(name="osbp", bufs=2) as osbp,
            tc.tile_pool(name="smallp", bufs=3) as smallp,
        ):
            # ---- weights + persistent activation tiles ---------------------
            wk_sb = [wpool.tile([128, DG], BF16, tag=f"wk{e}", name=f"wk{e}")
                     for e in range(ET)]
            wq_sb = [wpool.tile([128, DG], BF16, tag=f"wq{e}", name=f"wq{e}")
                     for e in range(ET)]
            wv_sb = [wpool.tile([128, DG], BF16, tag=f"wv{e}", name=f"wv{e}")
                     for e in range(ET)]
            wo_sb = [wpool.tile([128, D], BF16, tag=f"wo{p}", name=f"wo{p}")
                     for p in range(2)]

            kt = [proj.tile([128, S], BF16, tag=f"kt{p}", name=f"kt{p}")
                  for p in range(2)]
            qt = [proj.tile([128, S], BF16, tag=f"qt{p}", name=f"qt{p}")
                  for p in range(2)]
            # V in [j, head, dk+1] layout; the 65th column of ones makes the
            # AV matmul emit softmax denominators in PSUM row 64.
            v_sb = proj.tile([128, JT, HL, DK + 1], BF16, tag="v", name="v_sb")
            nc.vector.memset(v_sb[:, :, :, DK:DK + 1], 1.0)

            # Prime the ACT exp table set during the prologue so the first
            # real exp doesn't eat the ~2.7us ACT_TABLE_LOAD mid-pipeline.
            prime = wpool.tile([1, 8], F32, tag="prime", name="prime")
            nc.vector.memset(prime, 0.0)
            nc.scalar.activation(out=prime, in_=prime, func=EXP, scale=1.0)

            QS = (nc.sync, nc.scalar, nc.gpsimd)

            # ---- DMA emission ----------------------------------------------
            # Everything the first scores need goes first, interleaved so
            # neither gate (full xk for K-proj, xq chunk 0 for Q-proj) waits
            # behind the other: xk + xq-c0 + wk, then wq, xv, wv, wo, xq-c1.
            xk_t = [None] * ET
            xq_slab = [None] * QC
            rr = 0

            def load_xk(e):
                nonlocal rr
                xk_t[e] = xkp.tile([128, S], BF16, tag="xk", name=f"xk{e}")
                QS[rr % 3].dma_start(xk_t[e], xk[e * 128:(e + 1) * 128, :])
                rr += 1

            def load_xq(c, queues=(0, 1, 2)):
                xq_slab[c] = xqp.tile([128, ET, 512], BF16, tag="xqc",
                                      name=f"xq{c}")
                for e in range(ET):
                    QS[queues[e % len(queues)]].dma_start(
                        xq_slab[c][:, e, :],
                        xq[e * 128:(e + 1) * 128, c * 512:(c + 1) * 512])

            for e in range(3):
                load_xk(e)
            load_xq(0)
            for e in range(ET):
                QS[(rr + e) % 3].dma_start(
                    wk_sb[e], wk[e * 128:(e + 1) * 128, :])
            for e in range(3, ET):
                load_xk(e)
            for e in range(ET):
                QS[(rr + e) % 3].dma_start(
                    wq_sb[e], wq[e * 128:(e + 1) * 128, :])
            xv_t = [None] * ET
            for e in range(ET):
                xv_t[e] = xvp.tile([128, S], BF16, tag=f"xv{e}", name=f"xv{e}")
                QS[e % 3].dma_start(xv_t[e], xv[e * 128:(e + 1) * 128, :])
            for e in range(ET):
                QS[e % 3].dma_start(wv_sb[e], wv[e * 128:(e + 1) * 128, :])
            for p in range(2):
                QS[p % 3].dma_start(wo_sb[p], wo[p * 128:(p + 1) * 128, :])
            load_xq(1)

            # ---- prologue: K projection + Q chunk 0 ------------------------
            with tc.tile_pool(name="ps_a", bufs=8, space="PSUM") as ps_a:
                kacc = [ps_a.tile([128, 512], F32, tag="pa", name=f"kacc{i}")
                        for i in range(8)]
                for e in range(ET):
                    for p in range(2):
                        for cc in range(4):
                            nc.tensor.matmul(
                                kacc[p * 4 + cc],
                                wk_sb[e][:, p * 128:(p + 1) * 128],
                                xk_t[e][:, cc * 512:(cc + 1) * 512],
                                start=(e == 0), stop=(e == ET - 1),
                            )
                for p in range(2):
                    for cc in range(4):
                        nc.vector.tensor_copy(
                            kt[p][:, cc * 512:(cc + 1) * 512], kacc[p * 4 + cc])
                qacc = [ps_a.tile([128, 512], F32, tag="pa", name=f"qacc{p}")
                        for p in range(2)]
                for e in range(ET):
                    for p in range(2):
                        nc.tensor.matmul(
                            qacc[p],
                            wq_sb[e][:, p * 128:(p + 1) * 128],
                            xq_slab[0][:, e, :],
                            start=(e == 0), stop=(e == ET - 1),
                        )
                for p in range(2):
                    nc.vector.tensor_copy(qt[p][:, 0:512], qacc[p])

            # ---- phase B ---------------------------------------------------
            ex_tiles = {}
            av_state = {}
            outt = {}
            osb = {}

            with (
                tc.tile_pool(name="ps_sc", bufs=2, space="PSUM") as ps_sc,
                tc.tile_pool(name="ps_av", bufs=2, space="PSUM") as ps_av,
                tc.tile_pool(name="ps_fl", bufs=2, space="PSUM") as ps_fl,
            ):
                def emit_sc_jt(p, c, jt):
                    sc_t = ps_sc.tile([128, 2, 512], F32, tag="sc",
                                      name=f"sc{p}_{c}_{jt}")
                    csl = slice(c * 512, (c + 1) * 512)
                    jsl = slice(jt * 128, (jt + 1) * 128)
                    nc.tensor.matmul(sc_t[:, 0, :], kt[p][0:64, jsl],
                                     qt[p][0:64, csl], start=True, stop=True,
                                     tile_position=(0, 0))
                    nc.tensor.matmul(sc_t[:, 1, :], kt[p][64:128, jsl],
                                     qt[p][64:128, csl], start=True, stop=True,
                                     tile_position=(64, 0))
                    ex = expp.tile([128, 2, 512], BF16, tag="ex",
                                   name=f"ex{p}_{c}_{jt}")
                    nc.scalar.activation(out=ex, in_=sc_t, func=EXP,
                                         scale=SCALE)
                    ex_tiles[(p, c)].append(ex)

                def emit_av_jt(p, c, jt):
                    if jt == 0:
                        av_state[(p, c)] = (
                            ps_av.tile([128, 512], F32, tag="av",
                                       name=f"avA{p}_{c}"),
                            ps_av.tile([128, 512], F32, tag="av",
                                       name=f"avB{p}_{c}"),
                        )
                    avA, avB = av_state[(p, c)]
                    ex = ex_tiles[(p, c)][jt]
                    nc.tensor.matmul(avA[0:DK + 1, :], v_sb[:, jt, 2 * p, :],
                                     ex[:, 0, :],
                                     start=(jt == 0), stop=(jt == JT - 1))
                    nc.tensor.matmul(avB[0:DK + 1, :], v_sb[:, jt, 2 * p + 1, :],
                                     ex[:, 1, :],
                                     start=(jt == 0), stop=(jt == JT - 1))

                def emit_normalize(p, c):
                    # Evacuate both av banks immediately (rs = rowsum row,
                    # st = unnormalized outT) so the next slot's AV matmuls
                    # never wait on the reciprocal chain; then run the chain
                    # (DMA-fold to [4,128] so DVE reciprocal is 128-wide,
                    # ~0.85us instead of 3.2us on a single partition).
                    avA, avB = av_state[(p, c)]
                    ot = outtp.tile([128, 512], BF16, tag="ot",
                                    name=f"ot{p}_{c}")
                    rss, sts = [], []
                    for i, av_ in ((0, avA), (1, avB)):
                        rs = smallp.tile([1, 512], F32, tag="rs", bufs=2,
                                         name=f"rs{p}_{c}_{i}")
                        nc.vector.tensor_copy(rs, av_[DK:DK + 1, :])
                        st = smallp.tile([64, 512], F32, tag="st", bufs=2,
                                         name=f"st{p}_{c}_{i}")
                        nc.vector.tensor_copy(st, av_[0:DK, :])
                        rss.append(rs)
                        sts.append(st)
                    for i in range(2):
                        rs4 = smallp.tile([4, 128], F32, tag="rs4", bufs=2,
                                          name=f"rs4{p}_{c}_{i}")
                        nc.sync.dma_start(
                            rs4, rss[i].rearrange("one (pp f) -> one pp f",
                                                  pp=4))
                        nc.vector.reciprocal(rs4, rs4)
                        rsu = smallp.tile([1, 512], F32, tag="rsu", bufs=2,
                                          name=f"rsu{p}_{c}_{i}")
                        nc.sync.dma_start(
                            rsu.rearrange("one (pp f) -> one pp f", pp=4),
                            rs4)
                        rbc = smallp.tile([64, 512], F32, tag="rbc", bufs=2,
                                          name=f"rbc{p}_{c}_{i}")
                        nc.gpsimd.partition_broadcast(rbc, rsu)
                        nc.vector.tensor_tensor(
                            ot[i * 64:(i + 1) * 64, :], sts[i], rbc, MULT)
                    outt[(p, c)] = ot

                def emit_qproj(c):
                    qa = [ps_fl.tile([128, 512], F32, tag="fl",
                                     name=f"qa{c}_{p}") for p in range(2)]
                    for e in range(ET):
                        for p in range(2):
                            nc.tensor.matmul(
                                qa[p], wq_sb[e][:, p * 128:(p + 1) * 128],
                                xq_slab[c][:, e, :],
                                start=(e == 0), stop=(e == ET - 1))
                    for p in range(2):
                        nc.vector.tensor_copy(
                            qt[p][:, c * 512:(c + 1) * 512], qa[p])

                def emit_vq(i):
                    # quarter i: j-tiles 4i..4i+3 of V, projected [j, dg]
                    # via x-as-weights (no PE transposes needed). One j-tile
                    # per PSUM tile: a bank supports only one open
                    # accumulation group at a time.
                    for jj in range(4):
                        jt = 4 * i + jj
                        vt = ps_fl.tile([128, 512], F32, tag="fl",
                                        name=f"vq{i}_{jj}")
                        for e in range(ET):
                            nc.tensor.matmul(
                                vt[:, 0:DG],
                                xv_t[e][:, jt * 128:(jt + 1) * 128],
                                wv_sb[e],
                                start=(e == 0), stop=(e == ET - 1))
                        nc.vector.tensor_copy(
                            v_sb[:, jt, :, 0:DK],
                            vt[:, 0:DG].rearrange("p (h d) -> p h d", h=HL))

                def emit_wo_half(c, half):
                    if half == 0:
                        osb[c] = osbp.tile([128, 4, D], BF16, tag="osb",
                                           bufs=1, name=f"osb{c}")
                    ot0, ot1 = outt[(0, c)], outt[(1, c)]
                    for t4 in (2 * half, 2 * half + 1):
                        for ch in range(2):
                            wp = ps_fl.tile([128, 512], F32, tag="fl",
                                            name=f"wo{c}_{t4}_{ch}")
                            nc.tensor.matmul(
                                wp, ot0[:, t4 * 128:(t4 + 1) * 128],
                                wo_sb[0][:, ch * 512:(ch + 1) * 512],
                                start=True, stop=False)
                            nc.tensor.matmul(
                                wp, ot1[:, t4 * 128:(t4 + 1) * 128],
                                wo_sb[1][:, ch * 512:(ch + 1) * 512],
                                start=False, stop=True)
                            nc.vector.tensor_copy(
                                osb[c][:, t4, ch * 512:(ch + 1) * 512], wp)
                    dst = out[c * 512 + half * 256:
                              c * 512 + (half + 1) * 256, :].rearrange(
                        "(a j) e -> j a e", a=2)
                    nc.sync.dma_start(dst, osb[c][:, 2 * half:2 * half + 2, :])

                SLOTS = [(p, c) for c in range(QC) for p in range(2)]
                # Filler PE work per slot. Non-wo fillers go at jt 0/5/10
                # (they have no fresh dependencies); wo fillers go at jt 7/12
                # so the previous chunk's reciprocal chain (DVE/Pool) has
                # finished before the wo matmuls need outt.
                FILLERS = {
                    0: [("q", 1), ("v", 0), ("v", 1)],
                    1: [("v", 2), ("v", 3)],
                    2: [("q", 2)],
                    3: [("wo", 0, 0), ("wo", 0, 1)],
                    4: [("q", 3)],
                    5: [("wo", 1, 0)],
                    6: [("wo", 1, 1)],
                    7: [("wo", 2, 0), ("wo", 2, 1)],
                }

                def emit_filler(f):
                    if f[0] == "q":
                        emit_qproj(f[1])
                    elif f[0] == "v":
                        emit_vq(f[1])
                    else:
                        emit_wo_half(f[1], f[2])

                for s in range(9):
                    cur = SLOTS[s] if s < 8 else None
                    prev = SLOTS[s - 1] if s > 0 else None
                    fillers = list(FILLERS.get(s, []))
                    is_wo = fillers and fillers[0][0] == "wo"
                    spots = (7, 12) if is_wo else (0, 5, 10)
                    if cur is not None:
                        ex_tiles[cur] = []
                    # xq slab prefetch two chunks ahead (sync queue only;
                    # scalar queue must stay clear of waits once exp runs).
                    if s == 1:
                        load_xq(2, queues=(0,))
                    if s == 3:
                        load_xq(3, queues=(0,))
                    for jt in range(JT):
                        if cur is not None:
                            emit_sc_jt(*cur, jt)
                        if jt in spots and fillers:
                            emit_filler(fillers.pop(0))
                        # AV of the previous slot, shifted 2 j-tiles so the
                        # slot-start matmuls never wait on the previous
                        # slot's reciprocal chain freeing the av banks.
                        if prev is not None and jt >= 2:
                            emit_av_jt(*prev, jt - 2)
                    for f in fillers:
                        emit_filler(f)
                    if prev is not None:
                        emit_av_jt(*prev, 14)
                        emit_av_jt(*prev, 15)
                        emit_normalize(*prev)

                emit_wo_half(3, 0)
                emit_wo_half(3, 1)

    nc.compile()
    return nc


_NC = None


def _get_nc():
    global _NC
    if _NC is None:
        _NC = _build_nc()
    return _NC


def make_in_maps(query, key, value, Wq, Wk, Wv, Wo):
    query = np.ascontiguousarray(query, dtype=np.float32)
    key_ = np.ascontiguousarray(key, dtype=np.float32)
    value = np.ascontiguousarray(value, dtype=np.float32)
    xqT = [np.ascontiguousarray(query[b].T).astype(NPBF) for b in range(B)]
    xkT = [np.ascontiguousarray(key_[b].T).astype(NPBF) for b in range(B)]
    xvT = [np.ascontiguousarray(value[b].T).astype(NPBF) for b in range(B)]
    Wq = np.asarray(Wq, dtype=np.float32)
    Wk = np.asarray(Wk, dtype=np.float32)
    Wv = np.asarray(Wv, dtype=np.float32)
    Wo = np.asarray(Wo, dtype=np.float32)

    in_maps = []
    for core in range(8):
        b, g = divmod(core, 4)
        sl = slice(g * DG, (g + 1) * DG)
        in_maps.append({
            "xq": xqT[b],
            "xk": xkT[b],
            "xv": xvT[b],
            "wq": np.ascontiguousarray(Wq[:, sl]).astype(NPBF),
            "wk": np.ascontiguousarray(Wk[:, sl]).astype(NPBF),
            "wv": np.ascontiguousarray(Wv[:, sl]).astype(NPBF),
            "wo": np.ascontiguousarray(Wo[sl, :]).astype(NPBF),
        })
    return in_maps


def combine_results(results):
    out = np.zeros((B, S, D), dtype=np.float32)
    for core in range(8):
        out[core // 4] += np.asarray(results[core]["out"]).astype(np.float32)
    return out


def kernel(query, key, value, Wq, Wk, Wv, Wo, _trace=False):
    from concourse import bass_utils

    nc = _get_nc()
    in_maps = make_in_maps(query, key, value, Wq, Wk, Wv, Wo)
    r = bass_utils.run_bass_kernel_spmd(
        nc, in_maps, core_ids=list(range(8)), trace=_trace
    )
    kernel.last_results = r
    return combine_results(r.results)
